# revision 8
# baseline (speedup 1.0000x reference)
"""Trainium2 Bass kernel for the GNN attention-aggregator problem.

Strategy
--------
The reference computes, for every node, an attention-weighted aggregation
over incoming edges, then returns only `out[batch_idx]` (8192 rows).  Hence
only edges whose destination `r` is one of the unique batch nodes
(~7.9k of 100k; ~8% of the 1.6M edges) contribute to the output.

Per-core work (slot = destination node index within the unique batch set):
  - slots are bin-packed into 64 buckets = (8 cores x 8 slot-blocks of 128)
    balancing per-bucket edge counts; each core exclusively owns its slots,
    so NO cross-core communication is needed.
  - algebraic trick: with F[s] = sum_e e_val*feats[c_e] (segment-sum of raw
    feature rows) and D[s] = sum_e e_val,
        out[s] = (F[s] @ W + D[s]*b) / (D[s] + EPS)
    i.e. the W-projection is applied AFTER aggregation (U rows instead of E).
  - per-edge feature rows are fetched with hardware `dma_gather` from a
    host-compacted table (only rows the core actually references, keeping
    indices < 32768 for the int16 gather index format).
  - per-edge attention logits: s_dst via a fused multiply+reduce on VectorE,
    s_src via a 256B-row dma_gather from an on-device table; lrelu+exp on
    ScalarE; the weighted segment-sum runs on TensorE as a one-hot matmul
    accumulated in PSUM.

Host-side work is limited to sharding/layout: np.unique, edge filtering,
bucket assignment, index packing, and final row re-assembly `out_u[inv]`.
"""

import sys
import types

sys.path.insert(0, "/opt/trn_rl_repo")

import numpy as np


def _ensure_axon_hooks():
    # antenv.axon_hooks is absent in this image; bass_utils imports it when
    # trace=True.  Install a functional shim wired to the axon PJRT client.
    if "antenv.axon_hooks" in sys.modules:
        return
    try:
        import antenv
    except ImportError:
        return
    mod = types.ModuleType("antenv.axon_hooks")
    mod._hook = None
    mod.set_axon_ntff_profile_hook = lambda h: setattr(mod, "_hook", h)
    mod.get_axon_ntff_profile_hook = lambda: mod._hook
    sys.modules["antenv.axon_hooks"] = mod
    antenv.axon_hooks = mod
    try:
        from trn_agent_boot.trn_boot import _ntff_profile_via_ctypes

        mod.set_axon_ntff_profile_hook(
            _ntff_profile_via_ctypes("/opt/axon/libaxon_pjrt.so")
        )
    except Exception:
        pass


_ensure_axon_hooks()

import concourse.bacc as bacc
import concourse.mybir as mybir
import concourse.tile as tile
from concourse.bass_utils import run_bass_kernel_spmd

N_CORES = 8
DIN = 256
DOUT = 128
SLOPE = 0.1
EPS = 1e-8
NBLK = 8          # slot blocks per core (128 slots each)
SLOT_CAP = 128    # slots per block
USLOT = NBLK * SLOT_CAP
SSRC_PAD = 64     # f32 elems per row of the on-device s_src table (256B)

f32 = mybir.dt.float32
i16 = mybir.dt.int16


# ----------------------------------------------------------------------
# host-side sharding / layout
# ----------------------------------------------------------------------

def _pack_gather_idx(idx, width):
    """Pack an index list into the SWDGE layout: element i at
    [i % 16, i // 16], replicated across the 8 groups of 16 partitions."""
    n = len(idx)
    cols = width // 16
    out = np.zeros((16, cols), np.int16)
    out[np.arange(n) % 16, np.arange(n) // 16] = idx
    return np.tile(out, (8, 1))


def _prepare(feats, r, c, batch_idx):
    u, inv = np.unique(batch_idx, return_inverse=True)
    U = len(u)

    mask = np.isin(r, u)
    rf = r[mask].astype(np.int64)
    cf = c[mask].astype(np.int64)
    slot = np.searchsorted(u, rf)          # [Ef] in [0, U)

    # --- balance slots into 64 buckets (core, block), capacity 128 slots ---
    deg = np.bincount(slot, minlength=U)
    order = np.argsort(-deg, kind="stable")
    nb = N_CORES * NBLK
    load = np.zeros(nb, np.int64)
    fill = np.zeros(nb, np.int64)
    slot_bucket = np.empty(U, np.int64)
    slot_pos = np.empty(U, np.int64)
    for s in order:
        cand = np.where(fill < SLOT_CAP, load, np.iinfo(np.int64).max)
        bk = int(np.argmin(cand))
        slot_bucket[s] = bk
        slot_pos[s] = fill[bk]
        fill[bk] += 1
        load[bk] += deg[s]

    CB = max(1, int(np.ceil(load.max() / 128)))       # chunks per block
    NB = CB * 128                                     # padded edges per block

    e_bucket = slot_bucket[slot]
    e_core = e_bucket // NBLK
    e_block = e_bucket % NBLK
    e_slotg = e_block * SLOT_CAP + slot_pos[slot]     # [0, 1024)
    e_slotb = slot_pos[slot]                          # [0, 128) within block

    per_core = []
    Ts = []
    for k in range(N_CORES):
        km = e_core == k
        cf_k, blk_k = cf[km], e_block[km]
        slotg_k, slotb_k = e_slotg[km], e_slotb[km]

        own_slots = np.where(slot_bucket // NBLK == k)[0]
        u_nodes = u[own_slots]
        tbl_nodes = np.unique(np.concatenate([cf_k, u_nodes]))
        Ts.append(len(tbl_nodes))
        c_idx = np.searchsorted(tbl_nodes, cf_k)       # per-edge table idx

        eidx = np.zeros((NBLK, NB), np.int64)          # table idx (pad 0)
        egsl = np.zeros((NBLK, NB), np.int64)          # global-local slot (pad 0)
        ebsl = np.full((NBLK, NB), -1.0, np.float32)   # in-block slot (pad -1)
        for bl in range(NBLK):
            bm = blk_k == bl
            n = int(bm.sum())
            eidx[bl, :n] = c_idx[bm]
            egsl[bl, :n] = slotg_k[bm]
            ebsl[bl, :n] = slotb_k[bm]

        # u-gather index list: row (block*128+pos) -> table idx of that slot's node
        uidx = np.zeros(USLOT, np.int64)
        upos = (slot_bucket[own_slots] % NBLK) * SLOT_CAP + slot_pos[own_slots]
        uidx[upos] = np.searchsorted(tbl_nodes, u_nodes)
        per_core.append(
            dict(tbl_nodes=tbl_nodes, eidx=eidx, egsl=egsl, ebsl=ebsl, uidx=uidx)
        )

    T = int(max(Ts))
    meta = dict(
        u=u, inv=inv, U=U, CB=CB, NB=NB, T=T,
        slot_bucket=slot_bucket, slot_pos=slot_pos,
    )
    return per_core, meta


def _build_in_maps(feats, W, b, a, per_core, meta):
    T, NB, CB = meta["T"], meta["NB"], meta["CB"]
    NCHUNK = NBLK * CB
    E_pad = NBLK * NB

    # consts: [:,0:128] iota rows, [:,128:256] identity, [:,256] ones col,
    #         [:,257:385] ones rows
    consts = np.zeros((128, 385), np.float32)
    consts[:, 0:128] = np.arange(128, dtype=np.float32)[None, :]
    consts[:, 128:256] = np.eye(128, dtype=np.float32)
    consts[:, 256:385] = 1.0

    in_maps = []
    for k in range(N_CORES):
        pc = per_core[k]
        tbl = np.zeros((T, DIN), np.float32)
        tbl[: len(pc["tbl_nodes"])] = feats[pc["tbl_nodes"]]
        in_maps.append(
            {
                "table": tbl,
                "W": np.ascontiguousarray(W, np.float32),
                "W_T": np.ascontiguousarray(W.T, np.float32),
                "b_rep": np.ascontiguousarray(np.tile(b[None, :], (128, 1)), np.float32),
                "a_row": np.ascontiguousarray(a.reshape(1, 2 * DOUT), np.float32),
                "a_cols": np.ascontiguousarray(a.reshape(2, DOUT).T, np.float32),
                "consts": consts,
                "eidx": _pack_gather_idx(pc["eidx"].reshape(-1), E_pad),
                "egsl": _pack_gather_idx(pc["egsl"].reshape(-1), E_pad),
                "ebsl": np.ascontiguousarray(
                    pc["ebsl"].reshape(NBLK, CB, 128).transpose(2, 0, 1).reshape(128, NCHUNK)
                ),
                "uidx": _pack_gather_idx(pc["uidx"], USLOT),
            }
        )
    return in_maps


# ----------------------------------------------------------------------
# device program (SPMD, one NEFF for all 8 cores)
# ----------------------------------------------------------------------

def _build_program(T, NB, CB):
    NCHUNK = NBLK * CB
    E_pad = NBLK * NB

    nc = bacc.Bacc(None)
    d_table = nc.dram_tensor("table", [T, DIN], f32, kind="ExternalInput")
    d_W = nc.dram_tensor("W", [DIN, DOUT], f32, kind="ExternalInput")
    d_WT = nc.dram_tensor("W_T", [DOUT, DIN], f32, kind="ExternalInput")
    d_brep = nc.dram_tensor("b_rep", [128, DOUT], f32, kind="ExternalInput")
    d_arow = nc.dram_tensor("a_row", [1, 2 * DOUT], f32, kind="ExternalInput")
    d_acols = nc.dram_tensor("a_cols", [DOUT, 2], f32, kind="ExternalInput")
    d_consts = nc.dram_tensor("consts", [128, 385], f32, kind="ExternalInput")
    d_eidx = nc.dram_tensor("eidx", [128, E_pad // 16], i16, kind="ExternalInput")
    d_egsl = nc.dram_tensor("egsl", [128, E_pad // 16], i16, kind="ExternalInput")
    d_ebsl = nc.dram_tensor("ebsl", [128, NCHUNK], f32, kind="ExternalInput")
    d_uidx = nc.dram_tensor("uidx", [128, USLOT // 16], i16, kind="ExternalInput")
    d_ssrc = nc.dram_tensor("ssrc_tbl", [USLOT, SSRC_PAD], f32)     # internal
    d_out = nc.dram_tensor("out", [USLOT, DOUT], f32, kind="ExternalOutput")

    with tile.TileContext(nc) as tc:
        with (
            tc.tile_pool(name="const", bufs=1) as cpool,
            tc.tile_pool(name="gather", bufs=2) as gpool,
            tc.tile_pool(name="work", bufs=4) as wpool,
            tc.tile_pool(name="fsb", bufs=1) as fpool,
            tc.tile_pool(name="psA", bufs=2, space="PSUM") as psA,
            tc.tile_pool(name="psB", bufs=2, space="PSUM") as psB,
        ):
            # ---- load constants / small inputs ----
            t_consts = cpool.tile([128, 385], f32)
            nc.sync.dma_start(t_consts[:], d_consts[:])
            iota_rep = t_consts[:, 0:128]
            ident = t_consts[:, 128:256]
            ones_col = t_consts[:, 256:257]
            ones_row = t_consts[0:1, 257:385]          # [1, 128]

            t_W = cpool.tile([128, 2, DOUT], f32)
            nc.sync.dma_start(t_W[:], d_W.rearrange("(h p) d -> p h d", p=128))
            t_WT = cpool.tile([128, DIN], f32)
            nc.sync.dma_start(t_WT[:], d_WT[:])
            t_brep = cpool.tile([128, DOUT], f32)
            nc.sync.dma_start(t_brep[:], d_brep[:])
            t_arow = cpool.tile([1, 2 * DOUT], f32)
            nc.sync.dma_start(t_arow[:], d_arow[:])
            t_acols = cpool.tile([128, 2], f32)
            nc.sync.dma_start(t_acols[:], d_acols[:])

            t_eidx = cpool.tile([128, E_pad // 16], i16)
            nc.sync.dma_start(t_eidx[:], d_eidx[:])
            t_egsl = cpool.tile([128, E_pad // 16], i16)
            nc.sync.dma_start(t_egsl[:], d_egsl[:])
            t_ebsl = cpool.tile([128, NCHUNK], f32)
            nc.sync.dma_start(t_ebsl[:], d_ebsl[:])
            t_uidx = cpool.tile([128, USLOT // 16], i16)
            nc.sync.dma_start(t_uidx[:], d_uidx[:])

            # ---- w_src/w_dst = W @ a_src|a_dst replicated to 128 partitions ----
            # psum [128k-half, 2]: lhsT = W_T[:, half] (K=128 j, M=128 k), rhs=a_cols
            t_wsd = wpool.tile([128, 4], f32, tag="wsd")  # cols (h*2 + which)
            for h in range(2):
                ps_w = psB.tile([128, 2], f32, tag="pss")
                nc.tensor.matmul(
                    ps_w[:], t_WT[:, 128 * h : 128 * (h + 1)], t_acols[:],
                    start=True, stop=True,
                )
                nc.vector.tensor_copy(t_wsd[:, 2 * h : 2 * h + 2], ps_w[:])
            # transpose each column separately so each row lands at partition 0
            # t_wsd col j (j = h*2 + which): [128,1] -> [1,128]
            t_wrow_s = wpool.tile([1, DIN], f32, tag="wrow_s")
            t_wrow_d = wpool.tile([1, DIN], f32, tag="wrow_d")
            for h in range(2):
                for which in range(2):
                    ps_wT = psB.tile([1, 128], f32, tag="pss")
                    nc.tensor.transpose(ps_wT[:], t_wsd[:, 2 * h + which : 2 * h + which + 1], ident)
                    dst = t_wrow_s if which == 0 else t_wrow_d
                    nc.vector.tensor_copy(dst[0:1, 128 * h : 128 * (h + 1)], ps_wT[:])
            # replicate via K=1 ones matmul
            t_wrep = cpool.tile([128, 2 * DIN], f32)
            ps_rep = psB.tile([128, 2 * DIN], f32, tag="pss")
            nc.tensor.matmul(ps_rep[:, 0:DIN], ones_row, t_wrow_s[:],
                             start=True, stop=True)
            nc.tensor.matmul(ps_rep[:, DIN:], ones_row, t_wrow_d[:],
                             start=True, stop=True)
            nc.vector.tensor_copy(t_wrep[:], ps_rep[:])
            w_src_rep = t_wrep[:, 0:DIN]
            w_dst_rep = t_wrep[:, DIN:]

            # ---- c_both = b.(a_src + a_dst), replicated per partition ----
            t_cscr = wpool.tile([1, DOUT], f32, tag="cscr")
            nc.vector.tensor_tensor(
                t_cscr[:], t_arow[0:1, 0:DOUT], t_arow[0:1, DOUT : 2 * DOUT],
                mybir.AluOpType.add,
            )
            nc.vector.tensor_tensor(
                t_cscr[:], t_cscr[:], t_brep[0:1, :], mybir.AluOpType.mult
            )
            t_c1 = wpool.tile([1, 1], f32, tag="c1")
            nc.vector.tensor_reduce(
                t_c1[:], t_cscr[:], mybir.AxisListType.X, mybir.AluOpType.add
            )
            ps_crep = psB.tile([128, 1], f32, tag="pss")
            nc.tensor.matmul(ps_crep[:], ones_row, t_c1[:], start=True, stop=True)
            t_cboth = wpool.tile([128, 1], f32, tag="cboth")
            nc.vector.tensor_copy(t_cboth[:], ps_crep[:])

            # ---- s_src for own slots: gather u rows, project, store table ----
            t_vu = gpool.tile([128, USLOT // 128, DIN], f32, tag="vu")
            nc.gpsimd.dma_gather(t_vu[:], d_table[:], t_uidx[:], USLOT, USLOT, DIN)
            t_ssrc = wpool.tile([128, USLOT // 128], f32, tag="ssrc")
            for col in range(USLOT // 128):
                t_scrU = wpool.tile([128, DIN], f32, tag="scrU")
                nc.vector.tensor_tensor(
                    t_scrU[:], t_vu[:, col, :], w_src_rep, mybir.AluOpType.mult
                )
                nc.vector.tensor_reduce(
                    t_ssrc[:, col : col + 1], t_scrU[:],
                    mybir.AxisListType.X, mybir.AluOpType.add,
                )
            # fold both bias constants (b.a_src + b.a_dst) into the table
            nc.vector.tensor_scalar_add(t_ssrc[:], t_ssrc[:], t_cboth[:])
            # write to DRAM table (row = col*128 + p), col 0 of each row
            nc.sync.dma_start(
                d_ssrc.rearrange("(c p) f -> p c f", p=128)[:, :, 0],
                t_ssrc[:],
            )

            # ---- edge loop ----
            t_fsb = fpool.tile([128, NBLK, DIN + 1], f32)   # F blocks in SBUF
            for bl in range(NBLK):
                t_vf = gpool.tile([128, CB, DIN], f32, tag="vf")
                t_gs = gpool.tile([128, CB, SSRC_PAD], f32, tag="gs")
                # split gathers into <=1024-index calls (SWDGE per-inst limit)
                seg0 = 0
                while seg0 < CB:
                    seg1 = min(seg0 + 8, CB)
                    n = (seg1 - seg0) * 128
                    i0 = bl * (NB // 16) + seg0 * 8
                    nc.gpsimd.dma_gather(
                        t_vf[:, seg0:seg1, :], d_table[:],
                        t_eidx[:, i0 : i0 + n // 16], n, n, DIN,
                    )
                    nc.gpsimd.dma_gather(
                        t_gs[:, seg0:seg1, :], d_ssrc[:],
                        t_egsl[:, i0 : i0 + n // 16], n, n, SSRC_PAD,
                    )
                    seg0 = seg1
                ps_F = psA.tile([128, DIN], f32, tag="psF")
                ps_D = psA.tile([128, 1], f32, tag="psD")
                for j in range(CB):
                    ch = bl * CB + j
                    t_scr = wpool.tile([128, DIN], f32, tag="escr")
                    t_sd = wpool.tile([128, 1], f32, tag="esd")
                    nc.vector.tensor_tensor(
                        t_scr[:], t_vf[:, j, :], w_dst_rep, mybir.AluOpType.mult
                    )
                    nc.vector.tensor_reduce(
                        t_sd[:], t_scr[:], mybir.AxisListType.X, mybir.AluOpType.add
                    )
                    t_x = wpool.tile([128, 1], f32, tag="ex")
                    nc.vector.tensor_tensor(
                        t_x[:], t_gs[:, j, 0:1], t_sd[:], mybir.AluOpType.add
                    )
                    nc.vector.scalar_tensor_tensor(
                        t_x[:], t_x[:], SLOPE, t_x[:],
                        mybir.AluOpType.mult, mybir.AluOpType.max,
                    )
                    t_ev = wpool.tile([128, 1], f32, tag="eev")
                    nc.scalar.activation(
                        t_ev[:], t_x[:], mybir.ActivationFunctionType.Exp,
                    )
                    t_Aw = wpool.tile([128, 128], f32, tag="eAw")
                    nc.vector.tensor_scalar(
                        t_Aw[:], iota_rep, t_ebsl[:, ch : ch + 1], t_ev[:],
                        mybir.AluOpType.is_equal, mybir.AluOpType.mult,
                    )
                    nc.tensor.matmul(
                        ps_F[:], t_Aw[:], t_vf[:, j, :],
                        start=(j == 0), stop=(j == CB - 1),
                    )
                    nc.tensor.matmul(
                        ps_D[:], t_Aw[:], ones_col[:],
                        start=(j == 0), stop=(j == CB - 1),
                    )
                nc.vector.tensor_copy(t_fsb[:, bl, 0:DIN], ps_F[:])
                nc.vector.tensor_copy(t_fsb[:, bl, DIN : DIN + 1], ps_D[:])

            # ---- final projection per block ----
            for bl in range(NBLK):
                t_FT = wpool.tile([128, DIN], f32, tag="fFT")
                for h in range(2):
                    ps_T = psB.tile([128, 128], f32, tag="pss")
                    nc.tensor.transpose(
                        ps_T[:], t_fsb[:, bl, 128 * h : 128 * (h + 1)], ident
                    )
                    nc.vector.tensor_copy(t_FT[:, 128 * h : 128 * (h + 1)], ps_T[:])
                ps_o = psA.tile([128, DOUT], f32, tag="psF")
                for h in range(2):
                    nc.tensor.matmul(
                        ps_o[:], t_FT[:, 128 * h : 128 * (h + 1)], t_W[:, h, :],
                        start=(h == 0), stop=(h == 1),
                    )
                t_D = wpool.tile([128, 1], f32, tag="fD")
                nc.vector.tensor_scalar_add(t_D[:], t_fsb[:, bl, 256:257], EPS)
                t_rec = wpool.tile([128, 1], f32, tag="frec")
                nc.vector.reciprocal(t_rec[:], t_D[:])
                t_o = wpool.tile([128, DOUT], f32, tag="fo")
                nc.vector.scalar_tensor_tensor(
                    t_o[:], t_brep[:], t_fsb[:, bl, 256:257], ps_o[:],
                    mybir.AluOpType.mult, mybir.AluOpType.add,
                )
                nc.vector.tensor_scalar_mul(t_o[:], t_o[:], t_rec[:])
                nc.sync.dma_start(d_out[bl * 128 : (bl + 1) * 128, :], t_o[:])

    nc.finalize()
    return nc


# ----------------------------------------------------------------------
# entry point
# ----------------------------------------------------------------------

def run(feats, W, b, a, r, c, batch_idx, trace=False):
    feats = np.asarray(feats, np.float32)
    W = np.asarray(W, np.float32)
    b = np.asarray(b, np.float32)
    a = np.asarray(a, np.float32)
    r = np.asarray(r)
    c = np.asarray(c)
    batch_idx = np.asarray(batch_idx)

    per_core, meta = _prepare(feats, r, c, batch_idx)
    in_maps = _build_in_maps(feats, W, b, a, per_core, meta)
    nc = _build_program(meta["T"], meta["NB"], meta["CB"])
    res = run_bass_kernel_spmd(
        nc, in_maps, core_ids=list(range(N_CORES)), trace=trace
    )

    U, inv = meta["U"], meta["inv"]
    sb, sp = meta["slot_bucket"], meta["slot_pos"]
    out_u = np.empty((U, DOUT), np.float32)
    for k in range(N_CORES):
        own = np.where(sb // NBLK == k)[0]
        rows = (sb[own] % NBLK) * SLOT_CAP + sp[own]
        out_u[own] = res.results[k]["out"][rows]
    return out_u[inv], res


def kernel(feats, W, b, a, r, c, batch_idx):
    out, _ = run(feats, W, b, a, r, c, batch_idx)
    return out


if __name__ == "__main__":
    sys.path.insert(0, "/root/problem")
    import reference

    inputs = {k: np.asarray(v) for k, v in reference.setup_inputs().items()}
    expected = np.asarray(reference.reference(**inputs))
    actual = kernel(**inputs)
    denom = np.abs(expected).max() + 1e-30
    err = np.abs(actual - expected).max() / denom
    print("Relative error:", err)


# revision 12
# speedup vs baseline: 1.9403x; 1.9403x over previous
"""Trainium2 Bass kernel for the GNN attention-aggregator problem.

Strategy
--------
The reference computes, for every node, an attention-weighted aggregation
over incoming edges, then returns only `out[batch_idx]` (8192 rows).  Hence
only edges whose destination `r` is one of the unique batch nodes
(~7.9k of 100k; ~8% of the 1.6M edges) contribute to the output.

Per-core work (slot = destination node index within the unique batch set):
  - slots are bin-packed into 64 buckets = (8 cores x 8 slot-blocks of 128)
    balancing per-bucket edge counts; each core exclusively owns its slots,
    so NO cross-core communication is needed.
  - algebraic trick: with F[s] = sum_e e_val*feats[c_e] (segment-sum of raw
    feature rows) and D[s] = sum_e e_val,
        out[s] = (F[s] @ W + D[s]*b) / (D[s] + EPS)
    i.e. the W-projection is applied AFTER aggregation (U rows instead of E).
  - per-edge feature rows are fetched with hardware `dma_gather` from a
    host-compacted table (only rows the core actually references, keeping
    indices < 32768 for the int16 gather index format).
  - per-edge attention logits: s_dst via a fused multiply+reduce on VectorE,
    s_src via a 256B-row dma_gather from an on-device table; lrelu+exp on
    ScalarE; the weighted segment-sum runs on TensorE as a one-hot matmul
    accumulated in PSUM.

Host-side work is limited to sharding/layout: np.unique, edge filtering,
bucket assignment, index packing, and final row re-assembly `out_u[inv]`.
"""

import sys
import types

sys.path.insert(0, "/opt/trn_rl_repo")

import numpy as np


def _ensure_axon_hooks():
    # antenv.axon_hooks is absent in this image; bass_utils imports it when
    # trace=True.  Install a functional shim wired to the axon PJRT client.
    if "antenv.axon_hooks" in sys.modules:
        return
    try:
        import antenv
    except ImportError:
        return
    mod = types.ModuleType("antenv.axon_hooks")
    mod._hook = None
    mod.set_axon_ntff_profile_hook = lambda h: setattr(mod, "_hook", h)
    mod.get_axon_ntff_profile_hook = lambda: mod._hook
    sys.modules["antenv.axon_hooks"] = mod
    antenv.axon_hooks = mod
    try:
        from trn_agent_boot.trn_boot import _ntff_profile_via_ctypes

        mod.set_axon_ntff_profile_hook(
            _ntff_profile_via_ctypes("/opt/axon/libaxon_pjrt.so")
        )
    except Exception:
        pass


_ensure_axon_hooks()

import concourse.bacc as bacc
import concourse.mybir as mybir
import concourse.tile as tile
from concourse.bass_utils import run_bass_kernel_spmd

N_CORES = 8
DIN = 256
DOUT = 128
SLOPE = 0.1
EPS = 1e-8
NBLK = 8          # slot blocks per core (128 slots each)
SLOT_CAP = 128    # slots per block
USLOT = NBLK * SLOT_CAP
SSRC_PAD = 64     # f32 elems per row of the on-device s_src table (256B)

f32 = mybir.dt.float32
bf16 = mybir.dt.bfloat16
i16 = mybir.dt.int16


# ----------------------------------------------------------------------
# host-side sharding / layout
# ----------------------------------------------------------------------

def _pack_gather_idx(idx, width):
    """Pack an index list into the SWDGE layout: element i at
    [i % 16, i // 16], replicated across the 8 groups of 16 partitions."""
    n = len(idx)
    cols = width // 16
    out = np.zeros((16, cols), np.int16)
    out[np.arange(n) % 16, np.arange(n) // 16] = idx
    return np.tile(out, (8, 1))


def _prepare(feats, r, c, batch_idx):
    u, inv = np.unique(batch_idx, return_inverse=True)
    U = len(u)

    mask = np.isin(r, u)
    rf = r[mask].astype(np.int64)
    cf = c[mask].astype(np.int64)
    slot = np.searchsorted(u, rf)          # [Ef] in [0, U)

    # --- balance slots into 64 buckets (core, block), capacity 128 slots ---
    deg = np.bincount(slot, minlength=U)
    order = np.argsort(-deg, kind="stable")
    nb = N_CORES * NBLK
    load = np.zeros(nb, np.int64)
    fill = np.zeros(nb, np.int64)
    slot_bucket = np.empty(U, np.int64)
    slot_pos = np.empty(U, np.int64)
    for s in order:
        cand = np.where(fill < SLOT_CAP, load, np.iinfo(np.int64).max)
        bk = int(np.argmin(cand))
        slot_bucket[s] = bk
        slot_pos[s] = fill[bk]
        fill[bk] += 1
        load[bk] += deg[s]

    CB = max(1, int(np.ceil(load.max() / 128)))       # chunks per block
    NB = CB * 128                                     # padded edges per block

    e_bucket = slot_bucket[slot]
    e_core = e_bucket // NBLK
    e_block = e_bucket % NBLK
    e_slotg = e_block * SLOT_CAP + slot_pos[slot]     # [0, 1024)
    e_slotb = slot_pos[slot]                          # [0, 128) within block

    per_core = []
    Ts = []
    for k in range(N_CORES):
        km = e_core == k
        cf_k, blk_k = cf[km], e_block[km]
        slotg_k, slotb_k = e_slotg[km], e_slotb[km]

        own_slots = np.where(slot_bucket // NBLK == k)[0]
        u_nodes = u[own_slots]
        tbl_nodes = np.unique(np.concatenate([cf_k, u_nodes]))
        Ts.append(len(tbl_nodes))
        c_idx = np.searchsorted(tbl_nodes, cf_k)       # per-edge table idx

        # per-edge slot-node table index (for the s_src gather)
        u_tbl_idx = np.searchsorted(tbl_nodes, u_nodes)      # aligned w/ own_slots
        slot2tbl = np.zeros(NBLK * SLOT_CAP, np.int64)
        upos = (slot_bucket[own_slots] % NBLK) * SLOT_CAP + slot_pos[own_slots]
        slot2tbl[upos] = u_tbl_idx
        eidx = np.zeros((NBLK, NB), np.int64)          # table idx of c (pad 0)
        egsl = np.zeros((NBLK, NB), np.int64)          # table idx of slot node (pad 0)
        ebsl = np.full((NBLK, NB), -1.0, np.float32)   # in-block slot (pad -1)
        for bl in range(NBLK):
            bm = blk_k == bl
            n = int(bm.sum())
            eidx[bl, :n] = c_idx[bm]
            egsl[bl, :n] = slot2tbl[slotg_k[bm]]
            ebsl[bl, :n] = slotb_k[bm]

        per_core.append(
            dict(tbl_nodes=tbl_nodes, eidx=eidx, egsl=egsl, ebsl=ebsl)
        )

    T = int(-(-max(Ts) // 128) * 128)
    meta = dict(
        u=u, inv=inv, U=U, CB=CB, NB=NB, T=T,
        slot_bucket=slot_bucket, slot_pos=slot_pos,
    )
    return per_core, meta


def _build_in_maps(feats, W, b, a, per_core, meta):
    T, NB, CB = meta["T"], meta["NB"], meta["CB"]
    NCHUNK = NBLK * CB
    E_pad = NBLK * NB

    # consts: [:,0:128] iota rows, [:,128:256] identity, [:,256] ones col,
    #         [:,257:385] ones rows
    consts = np.zeros((128, 385), np.float32)
    consts[:, 0:128] = np.arange(128, dtype=np.float32)[None, :]
    consts[:, 128:256] = np.eye(128, dtype=np.float32)
    consts[:, 256:385] = 1.0

    import ml_dtypes
    bfnp = ml_dtypes.bfloat16
    iotab = np.zeros((128, 129), bfnp)
    iotab[:, 0:128] = np.arange(128, dtype=np.float32)[None, :].astype(bfnp)
    iotab[:, 128] = bfnp(1.0)

    in_maps = []
    for k in range(N_CORES):
        pc = per_core[k]
        nt = len(pc["tbl_nodes"])
        fb = feats[pc["tbl_nodes"]].astype(bfnp)          # [nt, 256]
        tbl = np.zeros((T, 384), bfnp)
        tbl[:nt, 0:DIN] = fb
        # transposed halves for the s-table matmul: [128 j, 2 h, T]
        tT = np.zeros((128, 2, T), bfnp)
        tT[:, 0, :nt] = fb[:, 0:128].T
        tT[:, 1, :nt] = fb[:, 128:256].T
        in_maps.append(
            {
                "table": tbl,
                "tT": tT,
                "iotab": iotab,
                "W": np.ascontiguousarray(W, np.float32),
                "W_T": np.ascontiguousarray(W.T, np.float32),
                "b_rep": np.ascontiguousarray(np.tile(b[None, :], (128, 1)), np.float32),
                "a_row": np.ascontiguousarray(a.reshape(1, 2 * DOUT), np.float32),
                "a_cols": np.ascontiguousarray(a.reshape(2, DOUT).T, np.float32),
                "consts": consts,
                "eidx": _pack_gather_idx(pc["eidx"].reshape(-1), E_pad),
                "egsl": _pack_gather_idx(pc["egsl"].reshape(-1), E_pad),
                "ebsl": np.ascontiguousarray(
                    pc["ebsl"].reshape(NBLK, CB, 128).transpose(2, 0, 1).reshape(128, NCHUNK)
                ),
            }
        )
    return in_maps


# ----------------------------------------------------------------------
# device program (SPMD, one NEFF for all 8 cores)
# ----------------------------------------------------------------------

def _build_program(T, NB, CB):
    NCHUNK = NBLK * CB
    E_pad = NBLK * NB
    TB = T // 128

    nc = bacc.Bacc(None, num_swdge_queues=4)
    d_table = nc.dram_tensor("table", [T, 384], bf16, kind="ExternalInput")
    d_tT = nc.dram_tensor("tT", [128, 2, T], bf16, kind="ExternalInput")
    d_iotab = nc.dram_tensor("iotab", [128, 129], bf16, kind="ExternalInput")
    d_W = nc.dram_tensor("W", [DIN, DOUT], f32, kind="ExternalInput")
    d_WT = nc.dram_tensor("W_T", [DOUT, DIN], f32, kind="ExternalInput")
    d_brep = nc.dram_tensor("b_rep", [128, DOUT], f32, kind="ExternalInput")
    d_arow = nc.dram_tensor("a_row", [1, 2 * DOUT], f32, kind="ExternalInput")
    d_acols = nc.dram_tensor("a_cols", [DOUT, 2], f32, kind="ExternalInput")
    d_consts = nc.dram_tensor("consts", [128, 385], f32, kind="ExternalInput")
    d_eidx = nc.dram_tensor("eidx", [128, E_pad // 16], i16, kind="ExternalInput")
    d_egsl = nc.dram_tensor("egsl", [128, E_pad // 16], i16, kind="ExternalInput")
    d_ebsl = nc.dram_tensor("ebsl", [128, NCHUNK], f32, kind="ExternalInput")
    d_out = nc.dram_tensor("out", [USLOT, DOUT], f32, kind="ExternalOutput")

    qn = [0]

    def next_q():
        qn[0] = (qn[0] + 1) % 4
        return qn[0]

    with tile.TileContext(nc) as tc:
        with (
            tc.tile_pool(name="const", bufs=1) as cpool,
            tc.tile_pool(name="gather", bufs=3) as gpool,
            tc.tile_pool(name="work", bufs=4) as wpool,
            tc.tile_pool(name="fsb", bufs=1) as fpool,
            tc.tile_pool(name="psA", bufs=2, space="PSUM") as psA,
            tc.tile_pool(name="psB", bufs=2, space="PSUM") as psB,
        ):
            # ---- constants / small inputs ----
            t_consts = cpool.tile([128, 385], f32)
            nc.sync.dma_start(t_consts[:], d_consts[:])
            ident = t_consts[:, 128:256]
            ones_row = t_consts[0:1, 257:385]          # [1, 128] f32

            t_iotab = cpool.tile([128, 129], bf16)
            nc.sync.dma_start(t_iotab[:], d_iotab[:])
            iota_b = t_iotab[:, 0:128]
            ones_b = t_iotab[:, 128:129]

            t_W = cpool.tile([128, 2, DOUT], f32)
            nc.sync.dma_start(t_W[:], d_W.rearrange("(h p) d -> p h d", p=128))
            t_WT = cpool.tile([128, DIN], f32)
            nc.sync.dma_start(t_WT[:], d_WT[:])
            t_brep = cpool.tile([128, DOUT], f32)
            nc.sync.dma_start(t_brep[:], d_brep[:])
            t_arow = cpool.tile([1, 2 * DOUT], f32)
            nc.sync.dma_start(t_arow[:], d_arow[:])
            t_acols = cpool.tile([128, 2], f32)
            nc.sync.dma_start(t_acols[:], d_acols[:])

            t_eidx = cpool.tile([128, E_pad // 16], i16)
            nc.sync.dma_start(t_eidx[:], d_eidx[:])
            t_egsl = cpool.tile([128, E_pad // 16], i16)
            nc.sync.dma_start(t_egsl[:], d_egsl[:])
            t_ebsl = cpool.tile([128, NCHUNK], f32)
            nc.sync.dma_start(t_ebsl[:], d_ebsl[:])

            # ---- w vectors: w_{src,dst} = W @ a_{src,dst} (per half), bf16 copy ----
            # order in t_wsd cols: (h*2 + which), which: 0=src, 1=dst
            t_wsd = wpool.tile([128, 4], f32, tag="wsd")
            for h in range(2):
                ps_w = psB.tile([128, 2], f32, tag="pss")
                nc.tensor.matmul(
                    ps_w[:], t_WT[:, 128 * h : 128 * (h + 1)], t_acols[:],
                    start=True, stop=True,
                )
                nc.vector.tensor_copy(t_wsd[:, 2 * h : 2 * h + 2], ps_w[:])
            # bf16 rhs for the s-table matmul, column order (dst, src)
            t_wb = wpool.tile([128, 2, 2], bf16, tag="wb")
            for h in range(2):
                nc.vector.tensor_copy(t_wb[:, h, 0:1], t_wsd[:, 2 * h + 1 : 2 * h + 2])
                nc.vector.tensor_copy(t_wb[:, h, 1:2], t_wsd[:, 2 * h : 2 * h + 1])
            # c-pair: (c_dst, c_src) = (b.a_dst, b.a_src), replicated to 128 partitions
            t_cpr = wpool.tile([1, 2], f32, tag="cpr")
            t_cscr = wpool.tile([1, DOUT], f32, tag="cscr")
            for which in range(2):
                nc.vector.tensor_tensor(
                    t_cscr[:], t_brep[0:1, :],
                    t_arow[0:1, DOUT * (1 - which) : DOUT * (2 - which)],
                    mybir.AluOpType.mult,
                )
                nc.vector.tensor_reduce(
                    t_cpr[0:1, which : which + 1], t_cscr[:],
                    mybir.AxisListType.X, mybir.AluOpType.add,
                )
            ps_crep = psB.tile([128, 2], f32, tag="pss")
            nc.tensor.matmul(ps_crep[:], ones_row, t_cpr[:], start=True, stop=True)
            t_crep = wpool.tile([128, 2], f32, tag="crep")
            nc.vector.tensor_copy(t_crep[:], ps_crep[:])

            # ---- s-table: s_dst/s_src per table row, written into table rows ----
            t_tT = cpool.tile([128, 2, T], bf16)
            nc.sync.dma_start(t_tT[:], d_tT[:])
            t_stbl = wpool.tile([128, TB, 2], f32, tag="stbl")
            for blk in range(TB):
                ps_s = psB.tile([128, 2], f32, tag="pss")
                for h in range(2):
                    nc.tensor.matmul(
                        ps_s[:], t_tT[:, h, blk * 128 : (blk + 1) * 128], t_wb[:, h, :],
                        start=(h == 0), stop=(h == 1),
                    )
                nc.vector.scalar_tensor_tensor(
                    t_stbl[:, blk, :], t_crep[:], 1.0, ps_s[:],
                    mybir.AluOpType.mult, mybir.AluOpType.add,
                )
            nc.sync.dma_start(
                d_table.rearrange("(c p) f -> p c f", p=128)[:, :, 256:260],
                t_stbl[:].bitcast(bf16),
            )

            # ---- edge loop ----
            t_fsb = fpool.tile([128, NBLK, DIN + 1], f32)   # F blocks + D col
            for bl in range(NBLK):
                t_vf = gpool.tile([128, CB, 384], bf16, tag="vf")
                t_gs = gpool.tile([128, CB, 128], bf16, tag="gs")
                seg0 = 0
                while seg0 < CB:
                    seg1 = min(seg0 + 8, CB)
                    n = (seg1 - seg0) * 128
                    i0 = bl * (NB // 16) + seg0 * 8
                    nc.gpsimd.dma_gather(
                        t_vf[:, seg0:seg1, :], d_table[:],
                        t_eidx[:, i0 : i0 + n // 16], n, n, 384,
                        queue_num=next_q(),
                    )
                    nc.gpsimd.dma_gather(
                        t_gs[:, seg0:seg1, :], d_table[:, 256:384],
                        t_egsl[:, i0 : i0 + n // 16], n, n, 128,
                        elem_step=384, queue_num=next_q(),
                    )
                    seg0 = seg1
                # per-block logits: x = lrelu(s_src[slot] + s_dst[c])
                vf32 = t_vf[:].bitcast(f32)        # [128, CB, 192]
                gs32 = t_gs[:].bitcast(f32)        # [128, CB, 64]
                t_x = wpool.tile([128, CB], f32, tag="ex")
                nc.vector.tensor_tensor(
                    t_x[:], gs32[:, :, 1], vf32[:, :, 128], mybir.AluOpType.add
                )
                nc.vector.scalar_tensor_tensor(
                    t_x[:], t_x[:], SLOPE, t_x[:],
                    mybir.AluOpType.mult, mybir.AluOpType.max,
                )
                ps_F = psA.tile([128, DIN], f32, tag="psF")
                ps_D = psA.tile([128, 1], f32, tag="psD")
                for j in range(CB):
                    ch = bl * CB + j
                    t_M = wpool.tile([128, 128], bf16, tag="eM")
                    nc.vector.tensor_scalar(
                        t_M[:], iota_b, t_ebsl[:, ch : ch + 1], 1.0,
                        mybir.AluOpType.is_equal, mybir.AluOpType.subtract,
                    )
                    t_Aw = wpool.tile([128, 128], bf16, tag="eAw")
                    nc.scalar.activation(
                        t_Aw[:], t_M[:], mybir.ActivationFunctionType.Exp,
                        bias=t_x[:, j : j + 1], scale=1.0e4,
                    )
                    nc.tensor.matmul(
                        ps_F[:], t_Aw[:], t_vf[:, j, 0:DIN],
                        start=(j == 0), stop=(j == CB - 1),
                    )
                    nc.tensor.matmul(
                        ps_D[:], t_Aw[:], ones_b,
                        start=(j == 0), stop=(j == CB - 1),
                    )
                nc.vector.tensor_copy(t_fsb[:, bl, 0:DIN], ps_F[:])
                nc.vector.tensor_copy(t_fsb[:, bl, DIN : DIN + 1], ps_D[:])

            # ---- final projection per block ----
            for bl in range(NBLK):
                t_FT = wpool.tile([128, DIN], f32, tag="fFT")
                for h in range(2):
                    ps_T = psB.tile([128, 128], f32, tag="pss")
                    nc.tensor.transpose(
                        ps_T[:], t_fsb[:, bl, 128 * h : 128 * (h + 1)], ident
                    )
                    nc.vector.tensor_copy(t_FT[:, 128 * h : 128 * (h + 1)], ps_T[:])
                ps_o = psA.tile([128, DOUT], f32, tag="psF")
                for h in range(2):
                    nc.tensor.matmul(
                        ps_o[:], t_FT[:, 128 * h : 128 * (h + 1)], t_W[:, h, :],
                        start=(h == 0), stop=(h == 1),
                    )
                t_D = wpool.tile([128, 1], f32, tag="fD")
                nc.vector.tensor_scalar_add(t_D[:], t_fsb[:, bl, 256:257], EPS)
                t_rec = wpool.tile([128, 1], f32, tag="frec")
                nc.vector.reciprocal(t_rec[:], t_D[:])
                t_o = wpool.tile([128, DOUT], f32, tag="fo")
                nc.vector.scalar_tensor_tensor(
                    t_o[:], t_brep[:], t_fsb[:, bl, 256:257], ps_o[:],
                    mybir.AluOpType.mult, mybir.AluOpType.add,
                )
                nc.vector.tensor_scalar_mul(t_o[:], t_o[:], t_rec[:])
                nc.sync.dma_start(d_out[bl * 128 : (bl + 1) * 128, :], t_o[:])

    nc.finalize()
    return nc


# ----------------------------------------------------------------------
# entry point
# ----------------------------------------------------------------------

def run(feats, W, b, a, r, c, batch_idx, trace=False):
    feats = np.asarray(feats, np.float32)
    W = np.asarray(W, np.float32)
    b = np.asarray(b, np.float32)
    a = np.asarray(a, np.float32)
    r = np.asarray(r)
    c = np.asarray(c)
    batch_idx = np.asarray(batch_idx)

    per_core, meta = _prepare(feats, r, c, batch_idx)
    in_maps = _build_in_maps(feats, W, b, a, per_core, meta)
    nc = _build_program(meta["T"], meta["NB"], meta["CB"])
    res = run_bass_kernel_spmd(
        nc, in_maps, core_ids=list(range(N_CORES)), trace=trace
    )

    U, inv = meta["U"], meta["inv"]
    sb, sp = meta["slot_bucket"], meta["slot_pos"]
    out_u = np.empty((U, DOUT), np.float32)
    for k in range(N_CORES):
        own = np.where(sb // NBLK == k)[0]
        rows = (sb[own] % NBLK) * SLOT_CAP + sp[own]
        out_u[own] = res.results[k]["out"][rows]
    return out_u[inv], res


def kernel(feats, W, b, a, r, c, batch_idx):
    out, _ = run(feats, W, b, a, r, c, batch_idx)
    return out


if __name__ == "__main__":
    sys.path.insert(0, "/root/problem")
    import reference

    inputs = {k: np.asarray(v) for k, v in reference.setup_inputs().items()}
    expected = np.asarray(reference.reference(**inputs))
    actual = kernel(**inputs)
    denom = np.abs(expected).max() + 1e-30
    err = np.abs(actual - expected).max() / denom
    print("Relative error:", err)


# revision 13
# speedup vs baseline: 2.1509x; 1.1085x over previous
"""Trainium2 Bass kernel for the GNN attention-aggregator problem.

Strategy
--------
The reference computes, for every node, an attention-weighted aggregation
over incoming edges, then returns only `out[batch_idx]` (8192 rows).  Hence
only edges whose destination `r` is one of the unique batch nodes
(~7.9k of 100k; ~8% of the 1.6M edges) contribute to the output.

Per-core work (slot = destination node index within the unique batch set):
  - slots are bin-packed into 64 buckets = (8 cores x 8 slot-blocks of 128)
    balancing per-bucket edge counts; each core exclusively owns its slots,
    so NO cross-core communication is needed.
  - algebraic trick: with F[s] = sum_e e_val*feats[c_e] (segment-sum of raw
    feature rows) and D[s] = sum_e e_val,
        out[s] = (F[s] @ W + D[s]*b) / (D[s] + EPS)
    i.e. the W-projection is applied AFTER aggregation (U rows instead of E).
  - per-edge feature rows are fetched with hardware `dma_gather` from a
    host-compacted table (only rows the core actually references, keeping
    indices < 32768 for the int16 gather index format).
  - per-edge attention logits: s_dst via a fused multiply+reduce on VectorE,
    s_src via a 256B-row dma_gather from an on-device table; lrelu+exp on
    ScalarE; the weighted segment-sum runs on TensorE as a one-hot matmul
    accumulated in PSUM.

Host-side work is limited to sharding/layout: np.unique, edge filtering,
bucket assignment, index packing, and final row re-assembly `out_u[inv]`.
"""

import sys
import types

sys.path.insert(0, "/opt/trn_rl_repo")

import numpy as np


def _ensure_axon_hooks():
    # antenv.axon_hooks is absent in this image; bass_utils imports it when
    # trace=True.  Install a functional shim wired to the axon PJRT client.
    if "antenv.axon_hooks" in sys.modules:
        return
    try:
        import antenv
    except ImportError:
        return
    mod = types.ModuleType("antenv.axon_hooks")
    mod._hook = None
    mod.set_axon_ntff_profile_hook = lambda h: setattr(mod, "_hook", h)
    mod.get_axon_ntff_profile_hook = lambda: mod._hook
    sys.modules["antenv.axon_hooks"] = mod
    antenv.axon_hooks = mod
    try:
        from trn_agent_boot.trn_boot import _ntff_profile_via_ctypes

        mod.set_axon_ntff_profile_hook(
            _ntff_profile_via_ctypes("/opt/axon/libaxon_pjrt.so")
        )
    except Exception:
        pass


_ensure_axon_hooks()

import concourse.bacc as bacc
import concourse.mybir as mybir
import concourse.tile as tile
from concourse.bass_utils import run_bass_kernel_spmd

N_CORES = 8
DIN = 256
DOUT = 128
SLOPE = 0.1
EPS = 1e-8
NBLK = 8          # slot blocks per core (128 slots each)
SLOT_CAP = 128    # slots per block
USLOT = NBLK * SLOT_CAP
SSRC_PAD = 64     # f32 elems per row of the on-device s_src table (256B)

f32 = mybir.dt.float32
bf16 = mybir.dt.bfloat16
i16 = mybir.dt.int16


# ----------------------------------------------------------------------
# host-side sharding / layout
# ----------------------------------------------------------------------

def _pack_gather_idx(idx, width):
    """Pack an index list into the SWDGE layout: element i at
    [i % 16, i // 16], replicated across the 8 groups of 16 partitions."""
    n = len(idx)
    cols = width // 16
    out = np.zeros((16, cols), np.int16)
    out[np.arange(n) % 16, np.arange(n) // 16] = idx
    return np.tile(out, (8, 1))


def _prepare(feats, r, c, batch_idx):
    u, inv = np.unique(batch_idx, return_inverse=True)
    U = len(u)

    mask = np.isin(r, u)
    rf = r[mask].astype(np.int64)
    cf = c[mask].astype(np.int64)
    slot = np.searchsorted(u, rf)          # [Ef] in [0, U)

    # --- balance slots into 64 buckets (core, block), capacity 128 slots ---
    deg = np.bincount(slot, minlength=U)
    order = np.argsort(-deg, kind="stable")
    nb = N_CORES * NBLK
    load = np.zeros(nb, np.int64)
    fill = np.zeros(nb, np.int64)
    slot_bucket = np.empty(U, np.int64)
    slot_pos = np.empty(U, np.int64)
    for s in order:
        cand = np.where(fill < SLOT_CAP, load, np.iinfo(np.int64).max)
        bk = int(np.argmin(cand))
        slot_bucket[s] = bk
        slot_pos[s] = fill[bk]
        fill[bk] += 1
        load[bk] += deg[s]

    CB = max(1, int(np.ceil(load.max() / 128)))       # chunks per block
    NB = CB * 128                                     # padded edges per block

    e_bucket = slot_bucket[slot]
    e_core = e_bucket // NBLK
    e_block = e_bucket % NBLK
    e_slotg = e_block * SLOT_CAP + slot_pos[slot]     # [0, 1024)
    e_slotb = slot_pos[slot]                          # [0, 128) within block

    per_core = []
    Ts = []
    for k in range(N_CORES):
        km = e_core == k
        cf_k, blk_k = cf[km], e_block[km]
        slotg_k, slotb_k = e_slotg[km], e_slotb[km]

        own_slots = np.where(slot_bucket // NBLK == k)[0]
        u_nodes = u[own_slots]
        tbl_nodes = np.unique(np.concatenate([cf_k, u_nodes]))
        Ts.append(len(tbl_nodes))
        c_idx = np.searchsorted(tbl_nodes, cf_k)       # per-edge table idx

        # per-edge slot-node table index (for the s_src gather)
        u_tbl_idx = np.searchsorted(tbl_nodes, u_nodes)      # aligned w/ own_slots
        slot2tbl = np.zeros(NBLK * SLOT_CAP, np.int64)
        upos = (slot_bucket[own_slots] % NBLK) * SLOT_CAP + slot_pos[own_slots]
        slot2tbl[upos] = u_tbl_idx
        eidx = np.zeros((NBLK, NB), np.int64)          # table idx of c (pad 0)
        egsl = np.zeros((NBLK, NB), np.int64)          # table idx of slot node (pad 0)
        ebsl = np.full((NBLK, NB), 1.0, np.float32)    # NEGATED in-block slot (pad -1 -> +1)
        for bl in range(NBLK):
            bm = blk_k == bl
            n = int(bm.sum())
            eidx[bl, :n] = c_idx[bm]
            egsl[bl, :n] = slot2tbl[slotg_k[bm]]
            ebsl[bl, :n] = -slotb_k[bm].astype(np.float32)

        per_core.append(
            dict(tbl_nodes=tbl_nodes, eidx=eidx, egsl=egsl, ebsl=ebsl)
        )

    T = int(-(-max(Ts) // 128) * 128)
    meta = dict(
        u=u, inv=inv, U=U, CB=CB, NB=NB, T=T,
        slot_bucket=slot_bucket, slot_pos=slot_pos,
    )
    return per_core, meta


def _build_in_maps(feats, W, b, a, per_core, meta):
    T, NB, CB = meta["T"], meta["NB"], meta["CB"]
    NCHUNK = NBLK * CB
    E_pad = NBLK * NB

    # consts: [:,0:128] iota rows, [:,128:256] identity, [:,256] ones col,
    #         [:,257:385] ones rows
    consts = np.zeros((128, 385), np.float32)
    consts[:, 0:128] = np.arange(128, dtype=np.float32)[None, :]
    consts[:, 128:256] = np.eye(128, dtype=np.float32)
    consts[:, 256:385] = 1.0

    import ml_dtypes
    bfnp = ml_dtypes.bfloat16
    iotab = np.zeros((128, 129), bfnp)
    iotab[:, 0:128] = np.arange(128, dtype=np.float32)[None, :].astype(bfnp)
    iotab[:, 128] = bfnp(1.0)

    in_maps = []
    for k in range(N_CORES):
        pc = per_core[k]
        nt = len(pc["tbl_nodes"])
        fb = feats[pc["tbl_nodes"]].astype(bfnp)          # [nt, 256]
        tbl = np.zeros((T, 384), bfnp)
        tbl[:nt, 0:DIN] = fb
        # transposed halves for the s-table matmul: [128 j, 2 h, T]
        tT = np.zeros((128, 2, T), bfnp)
        tT[:, 0, :nt] = fb[:, 0:128].T
        tT[:, 1, :nt] = fb[:, 128:256].T
        in_maps.append(
            {
                "table": tbl,
                "tT": tT,
                "iotab": iotab,
                "W": np.ascontiguousarray(W, np.float32),
                "W_T": np.ascontiguousarray(W.T, np.float32),
                "b_rep": np.ascontiguousarray(np.tile(b[None, :], (128, 1)), np.float32),
                "a_row": np.ascontiguousarray(a.reshape(1, 2 * DOUT), np.float32),
                "a_cols": np.ascontiguousarray(a.reshape(2, DOUT).T, np.float32),
                "consts": consts,
                "eidx": _pack_gather_idx(pc["eidx"].reshape(-1), E_pad),
                "egsl": _pack_gather_idx(pc["egsl"].reshape(-1), E_pad),
                "ebsl": np.ascontiguousarray(
                    pc["ebsl"].reshape(NBLK, CB, 128).transpose(2, 0, 1).reshape(128, NCHUNK)
                ),
            }
        )
    return in_maps


# ----------------------------------------------------------------------
# device program (SPMD, one NEFF for all 8 cores)
# ----------------------------------------------------------------------

def _build_program(T, NB, CB):
    NCHUNK = NBLK * CB
    E_pad = NBLK * NB
    TB = T // 128

    nc = bacc.Bacc(None, num_swdge_queues=4)
    d_table = nc.dram_tensor("table", [T, 384], bf16, kind="ExternalInput")
    d_tT = nc.dram_tensor("tT", [128, 2, T], bf16, kind="ExternalInput")
    d_iotab = nc.dram_tensor("iotab", [128, 129], bf16, kind="ExternalInput")
    d_W = nc.dram_tensor("W", [DIN, DOUT], f32, kind="ExternalInput")
    d_WT = nc.dram_tensor("W_T", [DOUT, DIN], f32, kind="ExternalInput")
    d_brep = nc.dram_tensor("b_rep", [128, DOUT], f32, kind="ExternalInput")
    d_arow = nc.dram_tensor("a_row", [1, 2 * DOUT], f32, kind="ExternalInput")
    d_acols = nc.dram_tensor("a_cols", [DOUT, 2], f32, kind="ExternalInput")
    d_consts = nc.dram_tensor("consts", [128, 385], f32, kind="ExternalInput")
    d_eidx = nc.dram_tensor("eidx", [128, E_pad // 16], i16, kind="ExternalInput")
    d_egsl = nc.dram_tensor("egsl", [128, E_pad // 16], i16, kind="ExternalInput")
    d_ebsl = nc.dram_tensor("ebsl", [128, NCHUNK], f32, kind="ExternalInput")
    d_out = nc.dram_tensor("out", [USLOT, DOUT], f32, kind="ExternalOutput")

    qn = [0]

    def next_q():
        qn[0] = (qn[0] + 1) % 4
        return qn[0]

    with tile.TileContext(nc) as tc:
        with (
            tc.tile_pool(name="const", bufs=1) as cpool,
            tc.tile_pool(name="gather", bufs=4) as gpool,
            tc.tile_pool(name="work", bufs=4) as wpool,
            tc.tile_pool(name="fsb", bufs=1) as fpool,
            tc.tile_pool(name="psA", bufs=2, space="PSUM") as psA,
            tc.tile_pool(name="psB", bufs=2, space="PSUM") as psB,
        ):
            # ---- constants / small inputs ----
            t_consts = cpool.tile([128, 385], f32)
            nc.sync.dma_start(t_consts[:], d_consts[:])
            ident = t_consts[:, 128:256]
            ones_row = t_consts[0:1, 257:385]          # [1, 128] f32

            t_iotab = cpool.tile([128, 129], bf16)
            nc.sync.dma_start(t_iotab[:], d_iotab[:])
            iota_b = t_iotab[:, 0:128]
            ones_b = t_iotab[:, 128:129]

            t_W = cpool.tile([128, 2, DOUT], f32)
            nc.sync.dma_start(t_W[:], d_W.rearrange("(h p) d -> p h d", p=128))
            t_WT = cpool.tile([128, DIN], f32)
            nc.sync.dma_start(t_WT[:], d_WT[:])
            t_brep = cpool.tile([128, DOUT], f32)
            nc.sync.dma_start(t_brep[:], d_brep[:])
            t_arow = cpool.tile([1, 2 * DOUT], f32)
            nc.sync.dma_start(t_arow[:], d_arow[:])
            t_acols = cpool.tile([128, 2], f32)
            nc.sync.dma_start(t_acols[:], d_acols[:])

            t_eidx = cpool.tile([128, E_pad // 16], i16)
            nc.sync.dma_start(t_eidx[:], d_eidx[:])
            t_egsl = cpool.tile([128, E_pad // 16], i16)
            nc.sync.dma_start(t_egsl[:], d_egsl[:])
            t_ebsl = cpool.tile([128, NCHUNK], f32)
            nc.sync.dma_start(t_ebsl[:], d_ebsl[:])

            # ---- w vectors: w_{src,dst} = W @ a_{src,dst} (per half), bf16 copy ----
            # order in t_wsd cols: (h*2 + which), which: 0=src, 1=dst
            t_wsd = wpool.tile([128, 4], f32, tag="wsd")
            for h in range(2):
                ps_w = psB.tile([128, 2], f32, tag="pss")
                nc.tensor.matmul(
                    ps_w[:], t_WT[:, 128 * h : 128 * (h + 1)], t_acols[:],
                    start=True, stop=True,
                )
                nc.vector.tensor_copy(t_wsd[:, 2 * h : 2 * h + 2], ps_w[:])
            # bf16 rhs for the s-table matmul, column order (dst, src)
            t_wb = wpool.tile([128, 2, 2], bf16, tag="wb")
            for h in range(2):
                nc.vector.tensor_copy(t_wb[:, h, 0:1], t_wsd[:, 2 * h + 1 : 2 * h + 2])
                nc.vector.tensor_copy(t_wb[:, h, 1:2], t_wsd[:, 2 * h : 2 * h + 1])
            # c-pair: (c_dst, c_src) = (b.a_dst, b.a_src), replicated to 128 partitions
            t_cpr = wpool.tile([1, 2], f32, tag="cpr")
            t_cscr = wpool.tile([1, DOUT], f32, tag="cscr")
            for which in range(2):
                nc.vector.tensor_tensor(
                    t_cscr[:], t_brep[0:1, :],
                    t_arow[0:1, DOUT * (1 - which) : DOUT * (2 - which)],
                    mybir.AluOpType.mult,
                )
                nc.vector.tensor_reduce(
                    t_cpr[0:1, which : which + 1], t_cscr[:],
                    mybir.AxisListType.X, mybir.AluOpType.add,
                )
            ps_crep = psB.tile([128, 2], f32, tag="pss")
            nc.tensor.matmul(ps_crep[:], ones_row, t_cpr[:], start=True, stop=True)
            t_crep = wpool.tile([128, 2], f32, tag="crep")
            nc.vector.tensor_copy(t_crep[:], ps_crep[:])

            # ---- s-table: s_dst/s_src per table row, written into table rows ----
            t_tT = cpool.tile([128, 2, T], bf16)
            nc.sync.dma_start(t_tT[:], d_tT[:])
            t_stbl = wpool.tile([128, TB, 2], f32, tag="stbl")
            for blk in range(TB):
                ps_s = psB.tile([128, 2], f32, tag="pss")
                for h in range(2):
                    nc.tensor.matmul(
                        ps_s[:], t_tT[:, h, blk * 128 : (blk + 1) * 128], t_wb[:, h, :],
                        start=(h == 0), stop=(h == 1),
                    )
                nc.vector.scalar_tensor_tensor(
                    t_stbl[:, blk, :], t_crep[:], 1.0, ps_s[:],
                    mybir.AluOpType.mult, mybir.AluOpType.add,
                )
            nc.sync.dma_start(
                d_table.rearrange("(c p) f -> p c f", p=128)[:, :, 256:260],
                t_stbl[:].bitcast(bf16),
            )

            # ---- edge loop ----
            t_fsb = fpool.tile([128, NBLK, DIN + 1], f32)   # F blocks + D col
            for bl in range(NBLK):
                t_vf = gpool.tile([128, CB, 384], bf16, tag="vf")
                t_gs = gpool.tile([128, CB, 128], bf16, tag="gs")
                seg0 = 0
                while seg0 < CB:
                    seg1 = min(seg0 + 8, CB)
                    n = (seg1 - seg0) * 128
                    i0 = bl * (NB // 16) + seg0 * 8
                    nc.gpsimd.dma_gather(
                        t_vf[:, seg0:seg1, :], d_table[:],
                        t_eidx[:, i0 : i0 + n // 16], n, n, 384,
                        queue_num=next_q(),
                    )
                    nc.gpsimd.dma_gather(
                        t_gs[:, seg0:seg1, :], d_table[:, 256:384],
                        t_egsl[:, i0 : i0 + n // 16], n, n, 128,
                        elem_step=384, queue_num=next_q(),
                    )
                    seg0 = seg1
                # per-block logits: x = lrelu(s_src[slot] + s_dst[c])
                vf32 = t_vf[:].bitcast(f32)        # [128, CB, 192]
                gs32 = t_gs[:].bitcast(f32)        # [128, CB, 64]
                t_x = wpool.tile([128, CB], f32, tag="ex")
                nc.vector.tensor_tensor(
                    t_x[:], gs32[:, :, 1], vf32[:, :, 128], mybir.AluOpType.add
                )
                nc.vector.scalar_tensor_tensor(
                    t_x[:], t_x[:], SLOPE, t_x[:],
                    mybir.AluOpType.mult, mybir.AluOpType.max,
                )
                ps_F = psA.tile([128, DIN], f32, tag="psF")
                ps_D = psA.tile([128, 1], f32, tag="psD")
                for j in range(CB):
                    ch = bl * CB + j
                    t_M = wpool.tile([128, 128], bf16, tag="eM")
                    nc.scalar.activation(
                        t_M[:], iota_b, mybir.ActivationFunctionType.Square,
                        bias=t_ebsl[:, ch : ch + 1], scale=1.0,
                    )
                    t_Aw = wpool.tile([128, 128], bf16, tag="eAw")
                    nc.scalar.activation(
                        t_Aw[:], t_M[:], mybir.ActivationFunctionType.Exp,
                        bias=t_x[:, j : j + 1], scale=-1.0e4,
                    )
                    nc.tensor.matmul(
                        ps_F[:], t_Aw[:], t_vf[:, j, 0:DIN],
                        start=(j == 0), stop=(j == CB - 1),
                    )
                    nc.tensor.matmul(
                        ps_D[:], t_Aw[:], ones_b,
                        start=(j == 0), stop=(j == CB - 1),
                    )
                nc.vector.tensor_copy(t_fsb[:, bl, 0:DIN], ps_F[:])
                nc.vector.tensor_copy(t_fsb[:, bl, DIN : DIN + 1], ps_D[:])

            # ---- final projection per block ----
            for bl in range(NBLK):
                t_FT = wpool.tile([128, DIN], f32, tag="fFT")
                for h in range(2):
                    ps_T = psB.tile([128, 128], f32, tag="pss")
                    nc.tensor.transpose(
                        ps_T[:], t_fsb[:, bl, 128 * h : 128 * (h + 1)], ident
                    )
                    nc.vector.tensor_copy(t_FT[:, 128 * h : 128 * (h + 1)], ps_T[:])
                ps_o = psA.tile([128, DOUT], f32, tag="psF")
                for h in range(2):
                    nc.tensor.matmul(
                        ps_o[:], t_FT[:, 128 * h : 128 * (h + 1)], t_W[:, h, :],
                        start=(h == 0), stop=(h == 1),
                    )
                t_D = wpool.tile([128, 1], f32, tag="fD")
                nc.vector.tensor_scalar_add(t_D[:], t_fsb[:, bl, 256:257], EPS)
                t_rec = wpool.tile([128, 1], f32, tag="frec")
                nc.vector.reciprocal(t_rec[:], t_D[:])
                t_o = wpool.tile([128, DOUT], f32, tag="fo")
                nc.vector.scalar_tensor_tensor(
                    t_o[:], t_brep[:], t_fsb[:, bl, 256:257], ps_o[:],
                    mybir.AluOpType.mult, mybir.AluOpType.add,
                )
                nc.vector.tensor_scalar_mul(t_o[:], t_o[:], t_rec[:])
                nc.sync.dma_start(d_out[bl * 128 : (bl + 1) * 128, :], t_o[:])

    nc.finalize()
    return nc


# ----------------------------------------------------------------------
# entry point
# ----------------------------------------------------------------------

def run(feats, W, b, a, r, c, batch_idx, trace=False):
    feats = np.asarray(feats, np.float32)
    W = np.asarray(W, np.float32)
    b = np.asarray(b, np.float32)
    a = np.asarray(a, np.float32)
    r = np.asarray(r)
    c = np.asarray(c)
    batch_idx = np.asarray(batch_idx)

    per_core, meta = _prepare(feats, r, c, batch_idx)
    in_maps = _build_in_maps(feats, W, b, a, per_core, meta)
    nc = _build_program(meta["T"], meta["NB"], meta["CB"])
    res = run_bass_kernel_spmd(
        nc, in_maps, core_ids=list(range(N_CORES)), trace=trace
    )

    U, inv = meta["U"], meta["inv"]
    sb, sp = meta["slot_bucket"], meta["slot_pos"]
    out_u = np.empty((U, DOUT), np.float32)
    for k in range(N_CORES):
        own = np.where(sb // NBLK == k)[0]
        rows = (sb[own] % NBLK) * SLOT_CAP + sp[own]
        out_u[own] = res.results[k]["out"][rows]
    return out_u[inv], res


def kernel(feats, W, b, a, r, c, batch_idx):
    out, _ = run(feats, W, b, a, r, c, batch_idx)
    return out


if __name__ == "__main__":
    sys.path.insert(0, "/root/problem")
    import reference

    inputs = {k: np.asarray(v) for k, v in reference.setup_inputs().items()}
    expected = np.asarray(reference.reference(**inputs))
    actual = kernel(**inputs)
    denom = np.abs(expected).max() + 1e-30
    err = np.abs(actual - expected).max() / denom
    print("Relative error:", err)


# revision 15
# speedup vs baseline: 2.6698x; 1.2413x over previous
"""Trainium2 Bass kernel for the GNN attention-aggregator problem.

Strategy
--------
The reference computes, for every node, an attention-weighted aggregation
over incoming edges, then returns only `out[batch_idx]` (8192 rows).  Hence
only edges whose destination `r` is one of the unique batch nodes
(~7.9k of 100k; ~8% of the 1.6M edges) contribute to the output.

Per-core work (slot = destination node index within the unique batch set):
  - slots are bin-packed into 64 buckets = (8 cores x 8 slot-blocks of 128)
    balancing per-bucket edge counts; each core exclusively owns its slots,
    so NO cross-core communication is needed.
  - algebraic trick: with F[s] = sum_e e_val*feats[c_e] (segment-sum of raw
    feature rows) and D[s] = sum_e e_val,
        out[s] = (F[s] @ W + D[s]*b) / (D[s] + EPS)
    i.e. the W-projection is applied AFTER aggregation (U rows instead of E).
  - per-edge feature rows are fetched with hardware `dma_gather` from a
    host-compacted table (only rows the core actually references, keeping
    indices < 32768 for the int16 gather index format).
  - per-edge attention logits: s_dst via a fused multiply+reduce on VectorE,
    s_src via a 256B-row dma_gather from an on-device table; lrelu+exp on
    ScalarE; the weighted segment-sum runs on TensorE as a one-hot matmul
    accumulated in PSUM.

Host-side work is limited to sharding/layout: np.unique, edge filtering,
bucket assignment, index packing, and final row re-assembly `out_u[inv]`.
"""

import sys
import types

sys.path.insert(0, "/opt/trn_rl_repo")

import numpy as np


def _ensure_axon_hooks():
    # antenv.axon_hooks is absent in this image; bass_utils imports it when
    # trace=True.  Install a functional shim wired to the axon PJRT client.
    if "antenv.axon_hooks" in sys.modules:
        return
    try:
        import antenv
    except ImportError:
        return
    mod = types.ModuleType("antenv.axon_hooks")
    mod._hook = None
    mod.set_axon_ntff_profile_hook = lambda h: setattr(mod, "_hook", h)
    mod.get_axon_ntff_profile_hook = lambda: mod._hook
    sys.modules["antenv.axon_hooks"] = mod
    antenv.axon_hooks = mod
    try:
        from trn_agent_boot.trn_boot import _ntff_profile_via_ctypes

        mod.set_axon_ntff_profile_hook(
            _ntff_profile_via_ctypes("/opt/axon/libaxon_pjrt.so")
        )
    except Exception:
        pass


_ensure_axon_hooks()

import concourse.bacc as bacc
import concourse.mybir as mybir
import concourse.tile as tile
from concourse.bass_utils import run_bass_kernel_spmd

N_CORES = 8
DIN = 256
DOUT = 128
SLOPE = 0.1
EPS = 1e-8
NBLK = 8          # slot blocks per core (128 slots each)
SLOT_CAP = 128    # slots per block
USLOT = NBLK * SLOT_CAP
SSRC_PAD = 64     # f32 elems per row of the on-device s_src table (256B)

f32 = mybir.dt.float32
bf16 = mybir.dt.bfloat16
i16 = mybir.dt.int16


# ----------------------------------------------------------------------
# host-side sharding / layout
# ----------------------------------------------------------------------

def _pack_gather_idx(idx, width):
    """Pack an index list into the SWDGE layout: element i at
    [i % 16, i // 16], replicated across the 8 groups of 16 partitions."""
    n = len(idx)
    cols = width // 16
    out = np.zeros((16, cols), np.int16)
    out[np.arange(n) % 16, np.arange(n) // 16] = idx
    return np.tile(out, (8, 1))


def _prepare(feats, r, c, batch_idx):
    u, inv = np.unique(batch_idx, return_inverse=True)
    U = len(u)

    mask = np.isin(r, u)
    rf = r[mask].astype(np.int64)
    cf = c[mask].astype(np.int64)
    slot = np.searchsorted(u, rf)          # [Ef] in [0, U)

    # --- balance slots into 64 buckets (core, block), capacity 128 slots ---
    deg = np.bincount(slot, minlength=U)
    order = np.argsort(-deg, kind="stable")
    nb = N_CORES * NBLK
    load = np.zeros(nb, np.int64)
    fill = np.zeros(nb, np.int64)
    slot_bucket = np.empty(U, np.int64)
    slot_pos = np.empty(U, np.int64)
    for s in order:
        cand = np.where(fill < SLOT_CAP, load, np.iinfo(np.int64).max)
        bk = int(np.argmin(cand))
        slot_bucket[s] = bk
        slot_pos[s] = fill[bk]
        fill[bk] += 1
        load[bk] += deg[s]

    CB = max(1, int(np.ceil(load.max() / 128)))       # chunks per block
    NB = CB * 128                                     # padded edges per block

    e_bucket = slot_bucket[slot]
    e_core = e_bucket // NBLK
    e_block = e_bucket % NBLK
    e_slotg = e_block * SLOT_CAP + slot_pos[slot]     # [0, 1024)
    e_slotb = slot_pos[slot]                          # [0, 128) within block

    per_core = []
    Ts = []
    for k in range(N_CORES):
        km = e_core == k
        cf_k, blk_k = cf[km], e_block[km]
        slotg_k, slotb_k = e_slotg[km], e_slotb[km]

        own_slots = np.where(slot_bucket // NBLK == k)[0]
        u_nodes = u[own_slots]
        tbl_nodes = np.unique(np.concatenate([cf_k, u_nodes]))
        Ts.append(len(tbl_nodes))
        c_idx = np.searchsorted(tbl_nodes, cf_k)       # per-edge table idx

        upos = (slot_bucket[own_slots] % NBLK) * SLOT_CAP + slot_pos[own_slots]
        eidx = np.zeros((NBLK, NB), np.int64)          # table idx of c (pad 0)
        egsl = np.zeros((NBLK, NB), np.int64)          # global-local slot (pad 0)
        ebsl = np.full((NBLK, NB), 1.0, np.float32)    # NEGATED in-block slot (pad -1 -> +1)
        for bl in range(NBLK):
            bm = blk_k == bl
            n = int(bm.sum())
            eidx[bl, :n] = c_idx[bm]
            egsl[bl, :n] = slotg_k[bm]
            ebsl[bl, :n] = -slotb_k[bm].astype(np.float32)

        per_core.append(
            dict(tbl_nodes=tbl_nodes, eidx=eidx, egsl=egsl, ebsl=ebsl,
                 u_nodes=u_nodes, upos=upos)
        )

    T = int(-(-max(Ts) // 128) * 128)
    meta = dict(
        u=u, inv=inv, U=U, CB=CB, NB=NB, T=T,
        slot_bucket=slot_bucket, slot_pos=slot_pos,
    )
    return per_core, meta


def _build_in_maps(feats, W, b, a, per_core, meta):
    T, NB, CB = meta["T"], meta["NB"], meta["CB"]
    NCHUNK = NBLK * CB
    E_pad = NBLK * NB

    # consts: [:,0:128] iota rows, [:,128:256] identity, [:,256] ones col,
    #         [:,257:385] ones rows
    consts = np.zeros((128, 385), np.float32)
    consts[:, 0:128] = np.arange(128, dtype=np.float32)[None, :]
    consts[:, 128:256] = np.eye(128, dtype=np.float32)
    consts[:, 256:385] = 1.0

    import ml_dtypes
    bfnp = ml_dtypes.bfloat16
    iotab = np.zeros((128, 129), bfnp)
    iotab[:, 0:128] = np.arange(128, dtype=np.float32)[None, :].astype(bfnp)
    iotab[:, 128] = bfnp(1.0)

    in_maps = []
    for k in range(N_CORES):
        pc = per_core[k]
        nt = len(pc["tbl_nodes"])
        fb = feats[pc["tbl_nodes"]].astype(bfnp)          # [nt, 256]
        tbl = np.zeros((T, DIN), bfnp)
        tbl[:nt] = fb
        # u-node feats transposed, arranged by slot position: [128 j, 2 h, USLOT]
        uT = np.zeros((128, 2, USLOT), bfnp)
        ufb = feats[pc["u_nodes"]].astype(bfnp)           # [n_u, 256]
        uT[:, 0, pc["upos"]] = ufb[:, 0:128].T
        uT[:, 1, pc["upos"]] = ufb[:, 128:256].T
        in_maps.append(
            {
                "table": tbl,
                "uT": uT,
                "iotab": iotab,
                "W": np.ascontiguousarray(W, np.float32),
                "W_T": np.ascontiguousarray(W.T, np.float32),
                "b_rep": np.ascontiguousarray(np.tile(b[None, :], (128, 1)), np.float32),
                "a_row": np.ascontiguousarray(a.reshape(1, 2 * DOUT), np.float32),
                "a_cols": np.ascontiguousarray(a.reshape(2, DOUT).T, np.float32),
                "consts": consts,
                "eidx": _pack_gather_idx(pc["eidx"].reshape(-1), E_pad),
                "egsl": _pack_gather_idx(pc["egsl"].reshape(-1), E_pad),
                "ebsl": np.ascontiguousarray(
                    pc["ebsl"].reshape(NBLK, CB, 128).transpose(2, 0, 1).reshape(128, NCHUNK)
                ),
            }
        )
    return in_maps


# ----------------------------------------------------------------------
# device program (SPMD, one NEFF for all 8 cores)
# ----------------------------------------------------------------------

def _build_program(T, NB, CB):
    NCHUNK = NBLK * CB
    E_pad = NBLK * NB

    nc = bacc.Bacc(None, num_swdge_queues=4)
    d_table = nc.dram_tensor("table", [T, DIN], bf16, kind="ExternalInput")
    d_uT = nc.dram_tensor("uT", [128, 2, USLOT], bf16, kind="ExternalInput")
    d_iotab = nc.dram_tensor("iotab", [128, 129], bf16, kind="ExternalInput")
    d_W = nc.dram_tensor("W", [DIN, DOUT], f32, kind="ExternalInput")
    d_WT = nc.dram_tensor("W_T", [DOUT, DIN], f32, kind="ExternalInput")
    d_brep = nc.dram_tensor("b_rep", [128, DOUT], f32, kind="ExternalInput")
    d_arow = nc.dram_tensor("a_row", [1, 2 * DOUT], f32, kind="ExternalInput")
    d_acols = nc.dram_tensor("a_cols", [DOUT, 2], f32, kind="ExternalInput")
    d_consts = nc.dram_tensor("consts", [128, 385], f32, kind="ExternalInput")
    d_eidx = nc.dram_tensor("eidx", [128, E_pad // 16], i16, kind="ExternalInput")
    d_egsl = nc.dram_tensor("egsl", [128, E_pad // 16], i16, kind="ExternalInput")
    d_ebsl = nc.dram_tensor("ebsl", [128, NCHUNK], f32, kind="ExternalInput")
    d_utab = nc.dram_tensor("utab", [USLOT, SSRC_PAD], f32)       # internal
    d_out = nc.dram_tensor("out", [USLOT, DOUT], f32, kind="ExternalOutput")

    qn = [0]

    def next_q():
        qn[0] = (qn[0] + 1) % 4
        return qn[0]

    with tile.TileContext(nc) as tc:
        with (
            tc.tile_pool(name="const", bufs=1) as cpool,
            tc.tile_pool(name="gather", bufs=4) as gpool,
            tc.tile_pool(name="work", bufs=4) as wpool,
            tc.tile_pool(name="fsb", bufs=1) as fpool,
            tc.tile_pool(name="psA", bufs=2, space="PSUM") as psA,
            tc.tile_pool(name="psB", bufs=2, space="PSUM") as psB,
        ):
            # ---- constants / small inputs ----
            t_consts = cpool.tile([128, 385], f32)
            nc.sync.dma_start(t_consts[:], d_consts[:])
            ident = t_consts[:, 128:256]
            ones_row = t_consts[0:1, 257:385]          # [1, 128] f32

            t_iotab = cpool.tile([128, 129], bf16)
            nc.sync.dma_start(t_iotab[:], d_iotab[:])
            iota_b = t_iotab[:, 0:128]
            ones_b = t_iotab[:, 128:129]

            t_W = cpool.tile([128, 2, DOUT], f32)
            nc.sync.dma_start(t_W[:], d_W.rearrange("(h p) d -> p h d", p=128))
            t_WT = cpool.tile([128, DIN], f32)
            nc.sync.dma_start(t_WT[:], d_WT[:])
            t_brep = cpool.tile([128, DOUT], f32)
            nc.sync.dma_start(t_brep[:], d_brep[:])
            t_arow = cpool.tile([1, 2 * DOUT], f32)
            nc.sync.dma_start(t_arow[:], d_arow[:])
            t_acols = cpool.tile([128, 2], f32)
            nc.sync.dma_start(t_acols[:], d_acols[:])

            t_eidx = cpool.tile([128, E_pad // 16], i16)
            nc.sync.dma_start(t_eidx[:], d_eidx[:])
            t_egsl = cpool.tile([128, E_pad // 16], i16)
            nc.sync.dma_start(t_egsl[:], d_egsl[:])
            t_ebsl = cpool.tile([128, NCHUNK], f32)
            nc.sync.dma_start(t_ebsl[:], d_ebsl[:])
            t_uT = cpool.tile([128, 2, USLOT], bf16)
            nc.sync.dma_start(t_uT[:], d_uT[:])

            # ---- w vectors: w_{src,dst} = W @ a_{src,dst} per half ----
            t_wsd = wpool.tile([128, 4], f32, tag="wsd")   # cols h*2+which (0=src,1=dst)
            for h in range(2):
                ps_w = psB.tile([128, 2], f32, tag="pss")
                nc.tensor.matmul(
                    ps_w[:], t_WT[:, 128 * h : 128 * (h + 1)], t_acols[:],
                    start=True, stop=True,
                )
                nc.vector.tensor_copy(t_wsd[:, 2 * h : 2 * h + 2], ps_w[:])
            # bf16 w_src halves (rhs of the u-table matmul)
            t_wsb = wpool.tile([128, 2], bf16, tag="wsb")
            nc.vector.tensor_copy(t_wsb[:, 0:1], t_wsd[:, 0:1])
            nc.vector.tensor_copy(t_wsb[:, 1:2], t_wsd[:, 2:3])
            # w_dst replicated to 128 partitions (bf16) via per-col transposes
            t_wdrow = wpool.tile([1, DIN], f32, tag="wdrow")
            for h in range(2):
                ps_wT = psB.tile([1, 128], f32, tag="pss")
                nc.tensor.transpose(ps_wT[:], t_wsd[:, 2 * h + 1 : 2 * h + 2], ident)
                nc.vector.tensor_copy(t_wdrow[0:1, 128 * h : 128 * (h + 1)], ps_wT[:])
            ps_rep = psB.tile([128, DIN], f32, tag="pss")
            nc.tensor.matmul(ps_rep[:], ones_row, t_wdrow[:], start=True, stop=True)
            t_wdrep = cpool.tile([128, DIN], bf16)
            nc.vector.tensor_copy(t_wdrep[:], ps_rep[:])

            # ---- c_both = b.(a_src + a_dst) replicated ----
            t_cscr = wpool.tile([1, DOUT], f32, tag="cscr")
            nc.vector.tensor_tensor(
                t_cscr[:], t_arow[0:1, 0:DOUT], t_arow[0:1, DOUT : 2 * DOUT],
                mybir.AluOpType.add,
            )
            nc.vector.tensor_tensor(
                t_cscr[:], t_cscr[:], t_brep[0:1, :], mybir.AluOpType.mult
            )
            t_c1 = wpool.tile([1, 1], f32, tag="c1")
            nc.vector.tensor_reduce(
                t_c1[:], t_cscr[:], mybir.AxisListType.X, mybir.AluOpType.add
            )
            ps_crep = psB.tile([128, 1], f32, tag="pss")
            nc.tensor.matmul(ps_crep[:], ones_row, t_c1[:], start=True, stop=True)
            t_crep = wpool.tile([128, 1], f32, tag="crep")
            nc.vector.tensor_copy(t_crep[:], ps_crep[:])

            # ---- u-table: s_src(+c_both) per slot, written to DRAM ----
            t_su = wpool.tile([128, NBLK], f32, tag="su")
            for bl in range(NBLK):
                ps_s = psB.tile([128, 1], f32, tag="pss")
                for h in range(2):
                    nc.tensor.matmul(
                        ps_s[:], t_uT[:, h, bl * 128 : (bl + 1) * 128],
                        t_wsb[:, h : h + 1],
                        start=(h == 0), stop=(h == 1),
                    )
                nc.vector.scalar_tensor_tensor(
                    t_su[:, bl : bl + 1], t_crep[:], 1.0, ps_s[:],
                    mybir.AluOpType.mult, mybir.AluOpType.add,
                )
            nc.sync.dma_start(
                d_utab.rearrange("(c p) f -> p c f", p=128)[:, :, 0], t_su[:]
            )

            # ---- edge loop ----
            t_fsb = fpool.tile([128, NBLK, DIN + 1], f32)   # F blocks + D col
            for bl in range(NBLK):
                t_vf = gpool.tile([128, CB, DIN], bf16, tag="vf")
                t_gs = gpool.tile([128, CB, SSRC_PAD], f32, tag="gs")
                seg0 = 0
                while seg0 < CB:
                    seg1 = min(seg0 + 8, CB)
                    n = (seg1 - seg0) * 128
                    i0 = bl * (NB // 16) + seg0 * 8
                    nc.gpsimd.dma_gather(
                        t_vf[:, seg0:seg1, :], d_table[:],
                        t_eidx[:, i0 : i0 + n // 16], n, n, DIN,
                        queue_num=next_q(),
                    )
                    nc.gpsimd.dma_gather(
                        t_gs[:, seg0:seg1, :], d_utab[:],
                        t_egsl[:, i0 : i0 + n // 16], n, n, SSRC_PAD,
                        queue_num=next_q(),
                    )
                    seg0 = seg1
                # per-chunk s_dst (fused mul+reduce), then block-batched logits
                t_sdb = wpool.tile([128, CB], f32, tag="esdb")
                t_scr = wpool.tile([128, DIN], bf16, tag="escr")
                for j in range(CB):
                    nc.vector.scalar_tensor_tensor(
                        t_scr[:], t_vf[:, j, :], 1.0, t_wdrep[:],
                        mybir.AluOpType.mult, mybir.AluOpType.mult,
                        accum_out=t_sdb[:, j : j + 1],
                    )
                t_x = wpool.tile([128, CB], f32, tag="ex")
                nc.vector.tensor_tensor(
                    t_x[:], t_gs[:, :, 0], t_sdb[:], mybir.AluOpType.add
                )
                nc.vector.scalar_tensor_tensor(
                    t_x[:], t_x[:], SLOPE, t_x[:],
                    mybir.AluOpType.mult, mybir.AluOpType.max,
                )
                ps_F = psA.tile([128, DIN], f32, tag="psF")
                ps_D = psA.tile([128, 1], f32, tag="psD")
                for j in range(CB):
                    ch = bl * CB + j
                    t_M = wpool.tile([128, 128], bf16, tag="eM")
                    nc.scalar.activation(
                        t_M[:], iota_b, mybir.ActivationFunctionType.Square,
                        bias=t_ebsl[:, ch : ch + 1], scale=1.0,
                    )
                    t_Aw = wpool.tile([128, 128], bf16, tag="eAw")
                    nc.scalar.activation(
                        t_Aw[:], t_M[:], mybir.ActivationFunctionType.Exp,
                        bias=t_x[:, j : j + 1], scale=-1.0e4,
                    )
                    nc.tensor.matmul(
                        ps_F[:], t_Aw[:], t_vf[:, j, :],
                        start=(j == 0), stop=(j == CB - 1),
                    )
                    nc.tensor.matmul(
                        ps_D[:], t_Aw[:], ones_b,
                        start=(j == 0), stop=(j == CB - 1),
                    )
                nc.vector.tensor_copy(t_fsb[:, bl, 0:DIN], ps_F[:])
                nc.vector.tensor_copy(t_fsb[:, bl, DIN : DIN + 1], ps_D[:])

            # ---- final projection per block ----
            for bl in range(NBLK):
                t_FT = wpool.tile([128, DIN], f32, tag="fFT")
                for h in range(2):
                    ps_T = psB.tile([128, 128], f32, tag="pss")
                    nc.tensor.transpose(
                        ps_T[:], t_fsb[:, bl, 128 * h : 128 * (h + 1)], ident
                    )
                    nc.vector.tensor_copy(t_FT[:, 128 * h : 128 * (h + 1)], ps_T[:])
                ps_o = psA.tile([128, DOUT], f32, tag="psF")
                for h in range(2):
                    nc.tensor.matmul(
                        ps_o[:], t_FT[:, 128 * h : 128 * (h + 1)], t_W[:, h, :],
                        start=(h == 0), stop=(h == 1),
                    )
                t_D = wpool.tile([128, 1], f32, tag="fD")
                nc.vector.tensor_scalar_add(t_D[:], t_fsb[:, bl, 256:257], EPS)
                t_rec = wpool.tile([128, 1], f32, tag="frec")
                nc.vector.reciprocal(t_rec[:], t_D[:])
                t_o = wpool.tile([128, DOUT], f32, tag="fo")
                nc.vector.scalar_tensor_tensor(
                    t_o[:], t_brep[:], t_fsb[:, bl, 256:257], ps_o[:],
                    mybir.AluOpType.mult, mybir.AluOpType.add,
                )
                nc.vector.tensor_scalar_mul(t_o[:], t_o[:], t_rec[:])
                nc.sync.dma_start(d_out[bl * 128 : (bl + 1) * 128, :], t_o[:])

    nc.finalize()
    return nc


# ----------------------------------------------------------------------
# entry point
# ----------------------------------------------------------------------

def run(feats, W, b, a, r, c, batch_idx, trace=False):
    feats = np.asarray(feats, np.float32)
    W = np.asarray(W, np.float32)
    b = np.asarray(b, np.float32)
    a = np.asarray(a, np.float32)
    r = np.asarray(r)
    c = np.asarray(c)
    batch_idx = np.asarray(batch_idx)

    per_core, meta = _prepare(feats, r, c, batch_idx)
    in_maps = _build_in_maps(feats, W, b, a, per_core, meta)
    nc = _build_program(meta["T"], meta["NB"], meta["CB"])
    res = run_bass_kernel_spmd(
        nc, in_maps, core_ids=list(range(N_CORES)), trace=trace
    )

    U, inv = meta["U"], meta["inv"]
    sb, sp = meta["slot_bucket"], meta["slot_pos"]
    out_u = np.empty((U, DOUT), np.float32)
    for k in range(N_CORES):
        own = np.where(sb // NBLK == k)[0]
        rows = (sb[own] % NBLK) * SLOT_CAP + sp[own]
        out_u[own] = res.results[k]["out"][rows]
    return out_u[inv], res


def kernel(feats, W, b, a, r, c, batch_idx):
    out, _ = run(feats, W, b, a, r, c, batch_idx)
    return out


if __name__ == "__main__":
    sys.path.insert(0, "/root/problem")
    import reference

    inputs = {k: np.asarray(v) for k, v in reference.setup_inputs().items()}
    expected = np.asarray(reference.reference(**inputs))
    actual = kernel(**inputs)
    denom = np.abs(expected).max() + 1e-30
    err = np.abs(actual - expected).max() / denom
    print("Relative error:", err)
